# revision 6
# baseline (speedup 1.0000x reference)
"""Distributed Trainium2 Bass kernel for nn_Attention_3453153706649.

Gated multi-head attention with pairwise bias:
  t = x @ w_proj.T -> split q,k,v per head; q *= hw^-0.5
  a = softmax(q k^T + bias)          (per batch, head)
  y = (sigmoid(x w_g^T + b_g) * (a v)) @ w_o.T + b_o
  returns (y, a.transpose(0,3,1,2))

Sharding: tensor-parallel over the 16 heads across 8 NeuronCores (2 heads
each); x replicated; bias / qkv / gate weights sharded by head; o-proj done
token-parallel after an AllToAll of the gated attention output.

All matmul inputs are staged in bf16 (host-converted); accumulation is fp32
in PSUM. Attention probabilities are produced transposed (S^T orientation:
key position on partitions) so the (B, Lk, H, Lq) attn output needs no
on-chip transposes and the P@V matmul gets its operands directly.
"""
import sys
sys.path.insert(0, "/opt/trn_rl_repo")

import numpy as np
import ml_dtypes

BF16 = ml_dtypes.bfloat16

B, L, E = 2, 1024, 1024
H, HW = 16, 64
NCORES = 8
HLOC = H // NCORES          # 2 heads per core
TOK = B * L                 # 2048 flattened tokens
TCH = TOK // NCORES         # 256 tokens per core for o-proj
KT = E // 128               # 8 contraction tiles


def build_graph(repeat: int = 1, norm_engine: str = "vector"):
    import concourse.bass as bass
    import concourse.mybir as mybir
    from concourse import bacc
    from concourse.tile import TileContext

    dt = mybir.dt
    nc = bacc.Bacc("TRN2", target_bir_lowering=False)

    # ---- external I/O ----------------------------------------------------
    xT_d = nc.declare_dram_parameter("xT", [KT, 128, TOK], dt.bfloat16, isOutput=False)
    wq_d = nc.declare_dram_parameter("wq", [128, KT, 128], dt.bfloat16, isOutput=False)
    wk_d = nc.declare_dram_parameter("wk", [128, KT, 128], dt.bfloat16, isOutput=False)
    wv_d = nc.declare_dram_parameter("wv", [128, KT, 128], dt.bfloat16, isOutput=False)
    wg_d = nc.declare_dram_parameter("wg", [128, KT, 128], dt.bfloat16, isOutput=False)
    wo_d = nc.declare_dram_parameter("wo", [128, KT, E], dt.bfloat16, isOutput=False)
    bg_d = nc.declare_dram_parameter("bg", [64, HLOC], dt.float32, isOutput=False)
    bo_d = nc.declare_dram_parameter("bo", [128, KT], dt.float32, isOutput=False)
    biasT_d = nc.declare_dram_parameter(
        "biasT", [B * HLOC, KT, 128, L], dt.float32, isOutput=False)

    attn_d = nc.declare_dram_parameter("attn", [B, L, HLOC, L], dt.bfloat16, isOutput=True)
    y_d = nc.declare_dram_parameter("y", [E, TCH], dt.float32, isOutput=True)

    # internal DRAM for the collective
    bounce = nc.dram_tensor("bounce", [NCORES, 128, TCH], dt.bfloat16)
    a2a_out = nc.dram_tensor("a2a_out", [NCORES, 128, TCH], dt.bfloat16)

    AF = mybir.ActivationFunctionType
    OP = mybir.AluOpType
    norm = nc.vector if norm_engine == "vector" else nc.gpsimd

    with TileContext(nc) as tc, \
         tc.tile_pool(name="persist", bufs=1) as P, \
         tc.tile_pool(name="epool", bufs=2) as EP, \
         tc.tile_pool(name="stream", bufs=3) as S, \
         tc.tile_pool(name="small", bufs=2) as SM, \
         tc.tile_pool(name="mm", bufs=2, space="PSUM") as MM, \
         tc.tile_pool(name="spsum", bufs=2, space="PSUM") as SP, \
         tc.tile_pool(name="ytpsum", bufs=1, space="PSUM") as YTP:

        for rep in range(repeat):
            # ---- load inputs ------------------------------------------------
            x_sb = [P.tile([128, TOK], dt.bfloat16, tag=f"x{kk}", name=f"x_sb{kk}") for kk in range(KT)]
            for kk in range(KT):
                nc.sync.dma_start(out=x_sb[kk], in_=xT_d[kk])
            wq_sb = P.tile([128, KT, 128], dt.bfloat16, tag="wq")
            wk_sb = P.tile([128, KT, 128], dt.bfloat16, tag="wk")
            wv_sb = P.tile([128, KT, 128], dt.bfloat16, tag="wv")
            wg_sb = P.tile([128, KT, 128], dt.bfloat16, tag="wg")
            wo_sb = P.tile([128, KT, E], dt.bfloat16, tag="wo")
            nc.sync.dma_start(out=wq_sb, in_=wq_d[:, :, :])
            nc.sync.dma_start(out=wk_sb, in_=wk_d[:, :, :])
            nc.sync.dma_start(out=wv_sb, in_=wv_d[:, :, :])
            nc.sync.dma_start(out=wg_sb, in_=wg_d[:, :, :])
            nc.sync.dma_start(out=wo_sb, in_=wo_d[:, :, :])
            bg_sb = P.tile([64, HLOC], dt.float32, tag="bg")
            bo_sb = P.tile([128, KT], dt.float32, tag="bo")
            nc.sync.dma_start(out=bg_sb, in_=bg_d[:, :])
            nc.sync.dma_start(out=bo_sb, in_=bo_d[:, :])

            # ---- phase 1: projections --------------------------------------
            q_sb = P.tile([128, TOK], dt.bfloat16, tag="q")   # [q_h0 ; q_h1]
            k_sb = P.tile([128, TOK], dt.bfloat16, tag="k")   # [k_h0 ; k_h1]
            for dst, wsb in ((q_sb, wq_sb), (k_sb, wk_sb)):
                for n in range(4):
                    ps = MM.tile([128, 512], dt.float32, tag="mm")
                    for kk in range(KT):
                        nc.tensor.matmul(ps, wsb[:, kk, :],
                                         x_sb[kk][:, n * 512:(n + 1) * 512],
                                         start=(kk == 0), stop=(kk == KT - 1))
                    nc.vector.tensor_copy(dst[:, n * 512:(n + 1) * 512], ps)

            # v natural (token-major), layout [v0(64) | 1 | v1(64) | 1]
            v_sb = [P.tile([128, 130], dt.bfloat16, tag=f"v{tt}", name=f"v_sb{tt}") for tt in range(16)]
            for tt in range(16):
                nc.vector.memset(v_sb[tt][:, 64:65], 1.0)
                nc.vector.memset(v_sb[tt][:, 129:130], 1.0)
                ps = MM.tile([128, 128], dt.float32, tag="mm")
                for kk in range(KT):
                    nc.tensor.matmul(ps, x_sb[kk][:, tt * 128:(tt + 1) * 128],
                                     wv_sb[:, kk, :],
                                     start=(kk == 0), stop=(kk == KT - 1))
                nc.scalar.copy(v_sb[tt][:, 0:64], ps[:, 0:64])
                nc.scalar.copy(v_sb[tt][:, 65:129], ps[:, 64:128])

            # gate: g = sigmoid(x w_g^T + b_g), per-head tiles (64, TOK)
            g_sb = [P.tile([64, TOK], dt.bfloat16, tag=f"g{h}", name=f"g_sb{h}") for h in range(HLOC)]
            for h in range(HLOC):
                for n in range(4):
                    ps = MM.tile([64, 512], dt.float32, tag="mm")
                    for kk in range(KT):
                        nc.tensor.matmul(ps, wg_sb[:, kk, h * 64:(h + 1) * 64],
                                         x_sb[kk][:, n * 512:(n + 1) * 512],
                                         start=(kk == 0), stop=(kk == KT - 1))
                    nc.scalar.activation(g_sb[h][:, n * 512:(n + 1) * 512], ps,
                                         AF.Sigmoid, bias=bg_sb[:, h:h + 1])

            # ---- phase 2: attention, one (batch, head) plane at a time -----
            yg_sb = [P.tile([64, L], dt.bfloat16, tag=f"yg{p}", name=f"yg_sb{p}") for p in range(B * HLOC)]
            for plane in range(B * HLOC):
                b, h = divmod(plane, HLOC)
                qs = q_sb[h * 64:(h + 1) * 64, b * L:(b + 1) * L]
                ks = k_sb[h * 64:(h + 1) * 64, b * L:(b + 1) * L]

                e_tiles = []
                yt_ps = YTP.tile([65, L], dt.float32, tag="yt")
                for t in range(8):
                    bias_t = S.tile([128, L], dt.float32, tag="bias")
                    nc.sync.dma_start(out=bias_t, in_=biasT_d[plane, t])
                    s_ps = SP.tile([128, L], dt.float32, tag="s")
                    e_t = EP.tile([128, L], dt.bfloat16, tag=f"e{t}")
                    for c in range(2):
                        sl = slice(c * 512, (c + 1) * 512)
                        nc.tensor.matmul(s_ps[:, sl], ks[:, t * 128:(t + 1) * 128],
                                         qs[:, sl], start=True, stop=True)
                        nc.vector.tensor_tensor(out=e_t[:, sl], in0=s_ps[:, sl],
                                                in1=bias_t[:, sl], op=OP.add)
                    nc.scalar.activation(e_t, e_t, AF.Exp)
                    vt = v_sb[b * 8 + t][:, h * 65:(h + 1) * 65]
                    for c in range(2):
                        sl = slice(c * 512, (c + 1) * 512)
                        nc.tensor.matmul(yt_ps[:, sl], vt, e_t[:, sl],
                                         start=(t == 0), stop=(t == 7))
                    e_tiles.append(e_t)

                # softmax denominators: row 64 of yt_ps -> reciprocal -> bcast
                sums_row = SM.tile([65, L], dt.float32, tag="sums")
                nc.scalar.copy(sums_row[64:65, :], yt_ps[64:65, :])
                rc_in = SM.tile([128, 8], dt.float32, tag="rcin")
                nc.sync.dma_start(out=rc_in, in_=sums_row[64:65, :])
                rc_out = SM.tile([128, 8], dt.bfloat16, tag="rcout")
                with nc.allow_low_precision(reason="bf16 softmax recip ok at 2e-2 tol"):
                    nc.vector.reciprocal(rc_out, rc_in)
                rb_row = SM.tile([1, L], dt.bfloat16, tag="rbrow")
                nc.sync.dma_start(out=rb_row, in_=rc_out)
                rbcast = SM.tile([128, L], dt.bfloat16, tag="rbcast")
                nc.gpsimd.partition_broadcast(rbcast, rb_row[0:1, :])

                # yg = (yt * r) * g   (rows 0:64)
                grh = SM.tile([64, L], dt.float32, tag="grh")
                nc.vector.tensor_tensor(out=grh, in0=rbcast[0:64, :],
                                        in1=g_sb[h][:, b * L:(b + 1) * L], op=OP.mult)
                nc.vector.tensor_tensor(out=yg_sb[plane], in0=yt_ps[0:64, :],
                                        in1=grh, op=OP.mult)
                for j in range(4):
                    nc.sync.dma_start(
                        out=bounce[4 * b + j, h * 64:(h + 1) * 64, :],
                        in_=yg_sb[plane][:, j * TCH:(j + 1) * TCH])

                # normalized transposed probabilities -> attn output
                for t in range(8):
                    at = S.tile([128, L], dt.bfloat16, tag="at")
                    norm.tensor_tensor(out=at, in0=e_tiles[t], in1=rbcast,
                                       op=OP.mult)
                    nc.sync.dma_start(
                        out=attn_d[b, t * 128:(t + 1) * 128, h, :], in_=at)

            # ---- phase 3: AllToAll + o-proj --------------------------------
            nc.gpsimd.collective_compute(
                "AllToAll", OP.bypass,
                replica_groups=[list(range(NCORES))],
                ins=[bounce[:, :, :]], outs=[a2a_out[:, :, :]])

            rb_sb = [P.tile([128, TCH], dt.bfloat16, tag=f"rb{kk}", name=f"rb_sb{kk}") for kk in range(KT)]
            for kk in range(KT):
                nc.sync.dma_start(out=rb_sb[kk], in_=a2a_out[kk])
            for mb in range(8):
                ps = MM.tile([128, TCH], dt.float32, tag="mm")
                for kk in range(KT):
                    nc.tensor.matmul(ps, wo_sb[:, kk, mb * 128:(mb + 1) * 128],
                                     rb_sb[kk], start=(kk == 0), stop=(kk == KT - 1))
                ys = SM.tile([128, TCH], dt.float32, tag="ys")
                nc.vector.tensor_scalar_add(ys, ps, bo_sb[:, mb:mb + 1])
                nc.sync.dma_start(out=y_d[mb * 128:(mb + 1) * 128, :], in_=ys)

    nc.finalize()
    return nc


# --------------------------------------------------------------------------
# host side
# --------------------------------------------------------------------------

def shard_inputs(x, bias, w_proj, w_o, b_o, w_g, b_g):
    """Build the per-core input dicts."""
    x = np.asarray(x, np.float32)
    bias = np.asarray(bias, np.float32)
    w_proj = np.asarray(w_proj, np.float32)
    w_o = np.asarray(w_o, np.float32)
    b_o = np.asarray(b_o, np.float32)
    w_g = np.asarray(w_g, np.float32)
    b_g = np.asarray(b_g, np.float32)

    xT = np.ascontiguousarray(x.reshape(TOK, E).T).reshape(KT, 128, TOK).astype(BF16)
    # w_proj rows are head-major blocks of [q(64); k(64); v(64)]
    wp = w_proj.reshape(H, 3 * HW, E)
    woT = np.ascontiguousarray(
        w_o.T.reshape(KT, 128, E).transpose(1, 0, 2)).astype(BF16)
    bo_h = np.ascontiguousarray(b_o.reshape(KT, 128).T)  # (128, KT)
    biasT_all = np.ascontiguousarray(bias.transpose(0, 3, 2, 1))  # (B, H, Lk, Lq)

    def wstack(mats):  # (128, E) -> (128, KT, 128) [p, kk, m]
        m = np.ascontiguousarray(np.concatenate(mats, axis=0).T)  # (E, 128)
        return np.ascontiguousarray(
            m.reshape(KT, 128, 128).transpose(1, 0, 2)).astype(BF16)

    in_maps = []
    for c in range(NCORES):
        hs = [HLOC * c + i for i in range(HLOC)]
        wq = wstack([wp[h, 0:64] * (HW ** -0.5) for h in hs])
        wk = wstack([wp[h, 64:128] for h in hs])
        wv = wstack([wp[h, 128:192] for h in hs])
        wg = wstack([w_g[c * 128 + i * 64: c * 128 + (i + 1) * 64] for i in range(2)])
        bg = np.ascontiguousarray(b_g[c * 128:(c + 1) * 128].reshape(HLOC, 64).T)
        biasT = np.stack([biasT_all[b, hs[h]].reshape(KT, 128, L)
                          for b in range(B) for h in range(HLOC)])
        in_maps.append({
            "xT": xT, "wq": wq, "wk": wk, "wv": wv, "wg": wg,
            "wo": woT, "bg": bg, "bo": bo_h,
            "biasT": np.ascontiguousarray(biasT),
        })
    return in_maps


def unshard_outputs(results):
    y_T = np.concatenate([results[c]["y"] for c in range(NCORES)], axis=1)  # (E, TOK)
    y = np.ascontiguousarray(y_T.T).reshape(B, L, E)
    attn = np.concatenate(
        [results[c]["attn"].astype(np.float32) for c in range(NCORES)], axis=2)
    return y, attn


class _Runner:
    """jit-once exec wrapper over the 8 axon TRN2 cores."""

    def __init__(self, nc):
        import jax
        import concourse.mybir as mybir
        from jax.sharding import Mesh, PartitionSpec, NamedSharding
        from jax.experimental.shard_map import shard_map
        from concourse.bass2jax import (_bass_exec_p, install_neuronx_cc_hook,
                                        partition_id_tensor)
        install_neuronx_cc_hook()
        self.jax = jax
        partition_name = (nc.partition_id_tensor.name
                          if nc.partition_id_tensor else None)
        in_names, out_names, out_avals, zero_outs = [], [], [], []
        for alloc in nc.m.functions[0].allocations:
            if not isinstance(alloc, mybir.MemoryLocationSet):
                continue
            name = alloc.memorylocations[0].name
            if alloc.kind == "ExternalInput":
                if name != partition_name:
                    in_names.append(name)
            elif alloc.kind == "ExternalOutput":
                shape = tuple(alloc.tensor_shape)
                npdt = mybir.dt.np(alloc.dtype)
                out_names.append(name)
                out_avals.append(jax.core.ShapedArray(shape, npdt))
                zero_outs.append(np.zeros(shape, npdt))
        self.in_names, self.out_names = in_names, out_names
        self.zero_outs = zero_outs
        n_params, n_outs = len(in_names), len(out_names)
        all_in = list(in_names) + list(out_names)
        if partition_name is not None:
            all_in.append(partition_name)

        def _body(*args):
            operands = list(args)
            if partition_name is not None:
                operands.append(partition_id_tensor())
            return tuple(_bass_exec_p.bind(
                *operands, out_avals=tuple(out_avals), in_names=tuple(all_in),
                out_names=tuple(out_names), lowering_input_output_aliases=(),
                sim_require_finite=False, sim_require_nnan=False, nc=nc))

        devices = jax.devices()[:NCORES]
        self.mesh = Mesh(np.asarray(devices), ("core",))
        self.sharding = NamedSharding(self.mesh, PartitionSpec("core"))
        self.fn = jax.jit(
            shard_map(_body, mesh=self.mesh,
                      in_specs=(PartitionSpec("core"),) * (n_params + n_outs),
                      out_specs=(PartitionSpec("core"),) * n_outs),
            donate_argnums=tuple(range(n_params, n_params + n_outs)),
            keep_unused=True)

    def put_inputs(self, in_maps):
        concat = [np.concatenate([np.asarray(in_maps[c][n]) for c in range(NCORES)],
                                 axis=0) for n in self.in_names]
        return [self.jax.device_put(a, self.sharding) for a in concat]

    def zero_dev(self):
        return [self.jax.device_put(
            np.concatenate([z] * NCORES, axis=0), self.sharding)
            for z in self.zero_outs]

    def exec(self, dev_in, dev_zeros):
        outs = self.fn(*dev_in, *dev_zeros)
        self.jax.block_until_ready(outs)
        return outs

    def run(self, in_maps):
        outs = self.exec(self.put_inputs(in_maps), self.zero_dev())
        np_outs = [np.asarray(o) for o in outs]
        per_core = []
        for c in range(NCORES):
            d = {}
            for name, arr, z in zip(self.out_names, np_outs, self.zero_outs):
                rows = z.shape[0]
                d[name] = arr[c * rows:(c + 1) * rows]
            per_core.append(d)
        return per_core


_CACHE = {}


def get_runner(repeat: int = 1, norm_engine: str = "vector"):
    key = (repeat, norm_engine)
    if key not in _CACHE:
        _CACHE[key] = _Runner(build_graph(repeat, norm_engine))
    return _CACHE[key]


def kernel(x, bias, w_proj, w_o, b_o, w_g, b_g):
    r = get_runner()
    in_maps = shard_inputs(x, bias, w_proj, w_o, b_o, w_g, b_g)
    results = r.run(in_maps)
    return unshard_outputs(results)
